# revision 22
# baseline (speedup 1.0000x reference)
"""Multi-head attention (B=4, T=2048, dim=2048, H=16, RoPE) on 8 TRN2 NeuronCores.

Tensor-parallel over heads: core c owns heads {2c, 2c+1} (projection dim
slice [256c, 256c+256)).  Each core computes q/k/v projections for its
heads, RoPE, full softmax attention for its 8 (batch, head) pairs, and a
partial output projection against its 256-row slice of wo; the host sums
the 8 bf16 partial outputs and adds wo_b.

Schedule: a 2-deep span pipeline.  While span i's S=QK^T matmuls and exp
(scalar engine, the per-span rate limiter) run, the PE is kept fed with
span i-1's PV matmuls, ao transposes and out-projection groups, pulled
from a work queue between S groups.  This holds across batch boundaries
(the projection phase of batch b+1 also pumps the queue), so the PE never
sees a sparse stretch and the HAM clock gate stays at full rate.

All matmuls run in bf16 with f32 PSUM accumulation; softmax runs exp in
f32->bf16 on the scalar engine with denominators accumulated via an extra
ones-column on V through the PV matmul.  RoPE runs on the vector engine
as full-128-partition ops; PV scaling (1/denominator) also on vector.
"""

import json
import sys
from collections import deque

sys.path.insert(0, "/opt/trn_rl_repo")

import ml_dtypes
import numpy as np

BF16 = ml_dtypes.bfloat16

# Problem shape (hardcoded per contract).
B, T, D = 4, 2048, 2048
H = 16
N_CORES = 8
HL = H // N_CORES  # heads per core = 2
DH = D // H  # head dim = 128
DOUT = HL * DH  # per-core projection width = 256
BT = B * T  # 8192 tokens
P = 128
NK = D // P  # 16 feature chunks
SPAN = 512
NSPAN = T // SPAN  # 4 token spans per batch
NTT = T // P  # 16 token tiles per batch
NKT2 = NTT // 2  # 8 k-tile pairs per batch


# ---------------------------------------------------------------------------
# BIR legalization: the walrus build in this container rejects instructions
# carrying more than one sync wait. Engines execute their stream in order, so
# hoisting excess waits into standalone EventSemaphore instructions directly
# before the instruction (same engine) is semantically equivalent; Tile's
# dependency graph is acyclic so this cannot deadlock.
# ---------------------------------------------------------------------------


def _legalize_waits(bir_json: bytes, max_inline: int = 1, es_capacity: int = 2):
    bir = json.loads(bir_json)
    for f in bir.get("functions", []):
        for bb in f.get("blocks", []):
            out = []
            for inst in bb.get("instructions", []):
                si = inst.get("sync_info")
                waits = (si or {}).get("on_wait") or []
                cap = (
                    es_capacity
                    if inst.get("opcode") == "EventSemaphore"
                    else max_inline
                )
                if len(waits) > cap:
                    keep, excess = waits[:cap], waits[cap:]
                    for ci in range(0, len(excess), es_capacity):
                        out.append(
                            {
                                "debug": inst.get("debug", 0),
                                "engine": inst["engine"],
                                "ins": [],
                                "name": f"{inst['name']}_xw{ci}",
                                "opcode": "EventSemaphore",
                                "outs": [],
                                "sync_info": {
                                    "on_update": [],
                                    "on_wait": excess[ci : ci + es_capacity],
                                },
                            }
                        )
                    si["on_wait"] = keep
                out.append(inst)
            bb["instructions"] = out
    return json.dumps(bir).encode()


_patched = False


def _install_compile_patch():
    global _patched
    if _patched:
        return
    _patched = True
    from concourse import bass2jax, bass_utils

    orig = bass_utils.compile_bir_kernel

    def patched_compile(bir_json, tmpdir, neff_name="file.neff"):
        return orig(_legalize_waits(bir_json), tmpdir, neff_name)

    bass2jax.compile_bir_kernel = patched_compile


# ---------------------------------------------------------------------------
# Kernel builder (one SPMD graph; per-core behavior differs only via inputs)
# ---------------------------------------------------------------------------


def _build_nc():
    import concourse.bass as bass
    import concourse.tile as tile
    from concourse import mybir
    from concourse.masks import make_identity

    f32 = mybir.dt.float32
    bf16 = mybir.dt.bfloat16

    nc = bass.Bass()
    xT = nc.declare_dram_parameter("xT", [D, BT], bf16, isOutput=False)
    wqT = nc.declare_dram_parameter("wqT", [D + 2, DOUT], bf16, isOutput=False)
    wkT = nc.declare_dram_parameter("wkT", [D + 2, DOUT], bf16, isOutput=False)
    wvT = nc.declare_dram_parameter("wvT", [D + 2, DOUT], bf16, isOutput=False)
    woT = nc.declare_dram_parameter("woT", [DOUT, D], bf16, isOutput=False)
    cosT = nc.declare_dram_parameter("cosT", [DH, T], bf16, isOutput=False)
    sinT = nc.declare_dram_parameter("sinT", [DH, T], bf16, isOutput=False)
    outp = nc.declare_dram_parameter("out", [BT, D], bf16, isOutput=True)

    HDH = DH + 1  # head slot width in v_ones (128 v cols + ones col)
    hh = DH // 2
    Copy = mybir.ActivationFunctionType.Copy
    Exp = mybir.ActivationFunctionType.Exp
    add = mybir.AluOpType.add
    mult = mybir.AluOpType.mult

    with tile.TileContext(nc) as tc:
        with (
            tc.tile_pool(name="wpool", bufs=1) as wpool,
            tc.tile_pool(name="xpool", bufs=3) as xpool,
            tc.tile_pool(name="qkT", bufs=1) as qkT,
            tc.tile_pool(name="vpool", bufs=2) as vpool,
            tc.tile_pool(name="aot", bufs=1) as aotp,
            tc.tile_pool(name="aoT", bufs=2) as aoTp,
            tc.tile_pool(name="epool", bufs=2) as epool,
            tc.tile_pool(name="misc", bufs=1) as misc,
            tc.tile_pool(name="recp", bufs=4) as recp,
            tc.tile_pool(name="obuf", bufs=3) as obuf,
            tc.tile_pool(name="psS", bufs=2, space="PSUM") as psS,
            tc.tile_pool(name="ps512", bufs=2, space="PSUM") as ps512,
            tc.tile_pool(name="pspv", bufs=2, space="PSUM") as pspv,
        ):
            # ---- persistent: weights, tables, bias columns ----
            ident = wpool.tile([P, P], bf16, tag="ident")

            def load_wT(name, dram):
                # two DMAs for the 16 k-chunks: [2048, DOUT] -> [128, 16, DOUT]
                wsb = wpool.tile([P, NK, DOUT], bf16, tag=name)
                wsrc = dram[:D, :].rearrange("(ko p) d -> p ko d", p=P)
                for c8 in range(2):
                    nc.gpsimd.dma_start(
                        out=wsb[:, c8 * 8 : (c8 + 1) * 8, :],
                        in_=wsrc[:, c8 * 8 : (c8 + 1) * 8, :],
                    )
                # biases: one DMA for all heads' normal + swapped columns
                # (layout [p, r*HL+m]: constant stride in (r m) order)
                bt = wpool.tile([DH, 2 * HL], bf16, tag=f"{name}bt")
                nc.gpsimd.dma_start(
                    out=bt,
                    in_=dram[D : D + 2, :].rearrange("r (m p) -> p (r m)", p=P),
                )
                bcols = [
                    (bt[:, m : m + 1], bt[:, HL + m : HL + m + 1])
                    for m in range(HL)
                ]
                return wsb, bcols

            wq_t, wq_bc = load_wT("wq", wqT)
            deferred = {}

            def load_rest():
                cos_sb = wpool.tile([DH, T], bf16, tag="cos")
                sin_sb = wpool.tile([DH, T], bf16, tag="sin")
                nc.gpsimd.dma_start(out=cos_sb, in_=cosT[:, :])
                nc.gpsimd.dma_start(out=sin_sb, in_=sinT[:, :])
                wk_t, wk_bc = load_wT("wk", wkT)
                wv_t, _ = load_wT("wv", wvT)
                # v bias broadcast tile [P, DOUT] from the wvT bias row
                vb_bc = wpool.tile([P, DOUT], bf16, tag="vb_bc")
                wvT_brow = wvT[D : D + 1, :]
                nc.gpsimd.dma_start(
                    out=vb_bc,
                    in_=bass.AP(
                        tensor=wvT_brow.tensor,
                        offset=wvT_brow.offset,
                        ap=[[0, P], wvT_brow.ap[-1]],
                    ),
                )
                deferred.update(
                    cos_sb=cos_sb, sin_sb=sin_sb, wk_t=wk_t, wk_bc=wk_bc,
                    wv_t=wv_t, vb_bc=vb_bc,
                )

            wo_t = []

            # ---- per-batch SBUF state (tags reused each batch) ----
            qT = [qkT.tile([P, T], bf16, tag=f"qT{m}", name=f"qT{m}") for m in range(HL)]
            kT = [qkT.tile([P, T], bf16, tag=f"kT{m}", name=f"kT{m}") for m in range(HL)]

            # -------------------------------------------------------------
            # Work queue: closures each emitting one short burst of PE work
            # (~0.3-1us).  pump(n) pops and emits up to n items.
            # -------------------------------------------------------------
            pending = deque()

            def pump(n):
                for _ in range(n):
                    if not pending:
                        return
                    pending.popleft()()

            op_alt = [0]

            def emit_outproj(b, s, tt, ds, aoT_s):
                # out[tokens tt-block, ds*512:+512] partial over this core's
                # two heads; psum -> bf16 staging -> DRAM
                t0 = b * T + s * SPAN + tt * P
                ps = ps512.tile([P, SPAN], f32, tag="p512", name="ps_op")
                for m in range(HL):
                    nc.tensor.matmul(
                        ps,
                        aoT_s[m][:, tt * P : (tt + 1) * P],
                        wo_t[m][:, ds * SPAN : (ds + 1) * SPAN],
                        start=(m == 0),
                        stop=(m == HL - 1),
                    )
                ob = obuf.tile([P, SPAN], bf16, tag="ob", name="ob")
                nc.vector.tensor_copy(out=ob, in_=ps)
                nc.sync.dma_start(
                    out=outp[t0 : t0 + P, ds * SPAN : (ds + 1) * SPAN], in_=ob
                )

            def emit_pv(m, tt, pvblk, ao_s, e_tiles, v_t):
                # PV for one 128-token q block: accumulate over all 16 k tiles
                po = pspv.tile([P, DH + 1], f32, tag="pv", name="po")
                for kt in range(NTT):
                    ek = e_tiles[m][kt // 2]
                    sl = slice(
                        (kt % 2) * SPAN + tt * P, (kt % 2) * SPAN + (tt + 1) * P
                    )
                    nc.tensor.matmul(
                        po,
                        ek[:, sl],
                        v_t[kt][:, m * HDH : (m + 1) * HDH],
                        start=(kt == 0),
                        stop=(kt == NTT - 1),
                    )
                rec = recp.tile([P, 1], f32, tag="rec", name="rec")
                nc.vector.reciprocal(rec, po[:, DH : DH + 1])
                nc.vector.tensor_scalar_mul(
                    ao_s[tt][:, m * DH : (m + 1) * DH], po[:, 0:DH], rec
                )

            def emit_transpose(m, tt, ptblk, ao_s, aoT_s):
                pt = pspv.tile([P, P], bf16, tag="pv", name="pt")
                nc.tensor.transpose(pt, ao_s[tt][:, m * DH : (m + 1) * DH], ident)
                nc.vector.tensor_copy(
                    out=aoT_s[m][:, tt * P : (tt + 1) * P], in_=pt
                )

            def enqueue_span(b, s, e_tiles, v_t):
                aoT_s = [aoTp.tile([P, SPAN], bf16, tag=f"aoT{m}", name=f"aoT{m}") for m in range(HL)]
                pvblk = None
                ptblk = None
                ao_s = [
                    aotp.tile([P, DOUT], bf16, tag=f"ao{tt}", name=f"ao{tt}")
                    for tt in range(SPAN // P)
                ]
                ntt = SPAN // P
                if b == B - 1 and s == NSPAN - 1:
                    for tt in range(ntt):
                        for m in range(HL):
                            pending.append(
                                lambda m=m, tt=tt: emit_pv(
                                    m, tt, pvblk, ao_s, e_tiles, v_t
                                )
                            )
                        for m in range(HL):
                            pending.append(
                                lambda m=m, tt=tt: emit_transpose(
                                    m, tt, ptblk, ao_s, aoT_s
                                )
                            )
                        for ds in range(D // SPAN):
                            pending.append(
                                lambda tt=tt, ds=ds: emit_outproj(
                                    b, s, tt, ds, aoT_s
                                )
                            )
                    return
                for tt in range(ntt):
                    for m in range(HL):
                        pending.append(
                            lambda m=m, tt=tt: emit_pv(m, tt, pvblk, ao_s, e_tiles, v_t)
                        )
                    if tt > 0:
                        for m in range(HL):
                            pending.append(
                                lambda m=m, tt=tt - 1: emit_transpose(
                                    m, tt, ptblk, ao_s, aoT_s
                                )
                            )
                        for ds in range(D // SPAN):
                            pending.append(
                                lambda tt=tt - 1, ds=ds: emit_outproj(
                                    b, s, tt, ds, aoT_s
                                )
                            )
                for m in range(HL):
                    pending.append(
                        lambda m=m: emit_transpose(m, ntt - 1, ptblk, ao_s, aoT_s)
                    )
                for ds in range(D // SPAN):
                    pending.append(
                        lambda ds=ds: emit_outproj(b, s, ntt - 1, ds, aoT_s)
                    )

            # -------------------------------------------------------------
            # Main emission
            # -------------------------------------------------------------
            for b in range(B):
                v_t = [
                    vpool.tile([P, HL * HDH], bf16, tag=f"v{tt}", name=f"v{tt}")
                    for tt in range(NTT)
                ]

                # ---- projection phase: QKV + RoPE, span pairs ----
                for s2 in range(NSPAN // 2):
                    xts = []
                    xsrcs = []
                    for half in range(2):
                        s = 2 * s2 + half
                        t0 = b * T + s * SPAN
                        xk = xpool.tile([P, NK, SPAN], bf16, tag="x", name="xk")
                        xsrcs.append(
                            xT[:, t0 : t0 + SPAN].rearrange("(ko p) t -> p ko t", p=P)
                        )
                        xts.append(xk)
                    # interleave halves so chunk k of both spans lands early
                    # (the k-loop consumes both halves per k)
                    for c8 in range(2):
                        for half in range(2):
                            nc.gpsimd.dma_start(
                                out=xts[half][:, c8 * 8 : (c8 + 1) * 8, :],
                                in_=xsrcs[half][:, c8 * 8 : (c8 + 1) * 8, :],
                            )
                    if not deferred:
                        load_rest()
                    cos_sb = deferred["cos_sb"]
                    sin_sb = deferred["sin_sb"]
                    wk_t = deferred["wk_t"]
                    wk_bc = deferred["wk_bc"]
                    wv_t = deferred["wv_t"]
                    vb_bc = deferred["vb_bc"]
                    sl2 = slice(2 * s2 * SPAN, (2 * s2 + 2) * SPAN)  # 1024 tokens
                    # q/k over the span pair: [128, 1024] psum, LDW shared
                    for dst, wsb, bcols in ((qT, wq_t, wq_bc), (kT, wk_t, wk_bc)):
                        for m in range(HL):
                            ps = psS.tile([P, 2 * SPAN], f32, tag="pS", name="ps_qk")
                            for k in range(NK):
                                for half in range(2):
                                    nc.tensor.matmul(
                                        ps[:, half * SPAN : (half + 1) * SPAN],
                                        wsb[:, k, m * P : (m + 1) * P],
                                        xts[half][:, k, :],
                                        start=(k == 0),
                                        stop=(k == NK - 1),
                                    )
                            # RoPE, full-partition ops; rotate-half swap is
                            # done in the PSUM-reading STTs (PSUM+SB pairs may
                            # differ in base partition; SB+SB may not), rotate
                            # sign folded into sinT ([-sin; +sin]),
                            # swapped-halves bias column:
                            #   tC       = (ps + b) * cosF
                            #   tS[0:64] = (ps[64:]+b_hi) * (-sin)
                            #   tS[64:]  = (ps[:64]+b_lo) * (+sin)
                            #   out = tC + tS
                            cs = cos_sb[:, sl2]
                            sn = sin_sb[:, sl2]
                            bc, bcs = bcols[m]
                            tC = misc.tile([P, 2 * SPAN], bf16, tag="rC", name="tC")
                            tS = misc.tile([P, 2 * SPAN], bf16, tag="rS", name="tS")
                            nc.vector.scalar_tensor_tensor(
                                tC, ps, bc, cs, add, mult
                            )
                            nc.vector.scalar_tensor_tensor(
                                tS[0:hh, :], ps[hh : 2 * hh, :], bcs[0:hh],
                                sn[0:hh, :], add, mult,
                            )
                            nc.vector.scalar_tensor_tensor(
                                tS[hh : 2 * hh, :], ps[0:hh, :], bcs[hh : 2 * hh],
                                sn[hh : 2 * hh, :], add, mult,
                            )
                            nc.vector.tensor_add(dst[m][:, sl2], tC, tS)

                    # v: per 128-token tile
                    for half in range(2):
                        s = 2 * s2 + half
                        for tt in range(SPAN // P):
                            gt = s * (SPAN // P) + tt
                            sl_p = slice(tt * P, (tt + 1) * P)
                            ps = ps512.tile([P, SPAN], f32, tag="p512", name="ps_v")
                            psv = ps[:, :DOUT]
                            for k in range(NK):
                                nc.tensor.matmul(
                                    psv,
                                    xts[half][:, k, sl_p],
                                    wv_t[:, k, :],
                                    start=(k == 0),
                                    stop=(k == NK - 1),
                                )
                            vt = v_t[gt]
                            ones_ap = bass.AP(
                                tensor=vt.tensor,
                                offset=vt.offset + DH,
                                ap=[vt.ap[0], [HDH, HL]],
                            )
                            nc.vector.memset(ones_ap, 1.0)
                            for m in range(HL):
                                nc.vector.tensor_add(
                                    vt[:, m * HDH : m * HDH + DH],
                                    psv[:, m * DH : (m + 1) * DH],
                                    vb_bc[:, m * DH : (m + 1) * DH],
                                )

                if b == 0:
                    # wo and the transpose identity are needed only from the
                    # first pumped out-proj/transpose items (during b=0
                    # attention); late position keeps them off the critical
                    # startup path.
                    make_identity(nc, ident)
                    for m in range(HL):
                        t = wpool.tile([P, D], bf16, tag=f"wo{m}")
                        nc.gpsimd.dma_start(out=t, in_=woT[m * P : (m + 1) * P, :])
                        wo_t.append(t)

                # ---- attention: S+exp per span, queue pumped between ----
                for s in range(NSPAN):
                    sl_q = slice(s * SPAN, (s + 1) * SPAN)
                    e_tiles = {m: [] for m in range(HL)}
                    for kt2 in range(NKT2):
                        for m in range(HL):
                            ps = psS.tile([P, 2 * SPAN], f32, tag="pS", name="ps_s")
                            for half in range(2):
                                nc.tensor.matmul(
                                    ps[:, half * SPAN : (half + 1) * SPAN],
                                    kT[m][
                                        :,
                                        (2 * kt2 + half) * P : (2 * kt2 + half + 1) * P,
                                    ],
                                    qT[m][:, sl_q],
                                    start=True,
                                    stop=True,
                                )
                            e = epool.tile(
                                [P, 2 * SPAN], bf16, tag=f"e{m}_{kt2}", name=f"e{m}"
                            )
                            nc.scalar.activation(out=e, in_=ps, func=Exp)
                            e_tiles[m].append(e)
                        pump(2)
                    # Drain all older-span items before enqueueing this span:
                    # keeps every reader of an e/aoT buffer version emitted
                    # before the next writer of that buffer (bufs=2 safety),
                    # and leaves exactly one span of filler in the queue.
                    pump(len(pending))
                    enqueue_span(b, s, e_tiles, v_t)

            # ---- drain remaining queued work (last span's PV/T/op) ----
            pump(len(pending))
    return nc


_nc_cache = None


def _get_nc():
    global _nc_cache
    if _nc_cache is None:
        _nc_cache = _build_nc()
    return _nc_cache


# ---------------------------------------------------------------------------
# Host wrapper
# ---------------------------------------------------------------------------


def _prep_inputs(x, pos, wq_w, wq_b, wk_w, wk_b, wv_w, wv_b, wo_w, wo_b):
    x2 = np.asarray(x, np.float32).reshape(BT, D)
    xT = np.ascontiguousarray(x2.T).astype(BF16)

    pos1 = np.asarray(pos, np.float32).reshape(T)
    freq = (1.0 / 10000.0 ** (np.arange(0, DH, 2, np.float32) / DH)).astype(np.float32)
    ang = pos1[None, :] * freq[:, None]  # [64, T]
    cos1 = np.cos(ang).astype(BF16)
    sin1 = np.sin(ang).astype(BF16)
    # duplicate across both dh halves -> [128, T]; sin carries the
    # rotate-half sign: rows 0:64 = -sin, rows 64:128 = +sin
    cosT = np.concatenate([cos1, cos1], axis=0)
    sinT = np.concatenate([-sin1, sin1], axis=0)

    scale = np.float32(1.0 / np.sqrt(DH))

    def wslice(w, bvec, c, s=None, swap_row=False):
        w = np.asarray(w, np.float32)
        bvec = np.asarray(bvec, np.float32)
        ws = w[c * DOUT : (c + 1) * DOUT]  # [256, D]
        bs = bvec[c * DOUT : (c + 1) * DOUT]
        if s is not None:
            ws = ws * s
            bs = bs * s
        rows = D + 2 if swap_row else D + 1
        out = np.empty((rows, DOUT), BF16)
        out[:D] = ws.T.astype(BF16)
        out[D] = bs.astype(BF16)
        if swap_row:
            # per-head swapped dh halves of the bias
            bsw = bs.reshape(HL, 2, DH // 2)[:, ::-1, :].reshape(DOUT)
            out[D + 1] = bsw.astype(BF16)
        return out

    in_maps = []
    for c in range(N_CORES):
        woTc = (
            np.asarray(wo_w, np.float32)[:, c * DOUT : (c + 1) * DOUT]
            .T.astype(BF16)
            .copy()
        )
        in_maps.append(
            {
                "xT": xT,
                "wqT": wslice(wq_w, wq_b, c, scale, swap_row=True),
                "wkT": wslice(wk_w, wk_b, c, swap_row=True),
                "wvT": wslice(wv_w, wv_b, c, swap_row=True),
                "woT": woTc,
                "cosT": cosT,
                "sinT": sinT,
            }
        )
    return in_maps


def _run(in_maps, trace=False):
    _install_compile_patch()
    from concourse.bass_utils import run_bass_kernel_spmd

    nc = _get_nc()
    return run_bass_kernel_spmd(
        nc, in_maps, core_ids=list(range(N_CORES)), trace=trace
    )


def kernel(**inputs):
    inputs = {k: np.asarray(v) for k, v in inputs.items()}
    in_maps = _prep_inputs(**inputs)
    r = _run(in_maps, trace=False)
    acc = np.zeros((BT, D), np.float32)
    for c in range(N_CORES):
        acc += r.results[c]["out"].astype(np.float32)
    acc += np.asarray(inputs["wo_b"], np.float32)
    return acc.reshape(B, T, D)


# revision 23
# speedup vs baseline: 1.1950x; 1.1950x over previous
"""Multi-head attention (B=4, T=2048, dim=2048, H=16, RoPE) on 8 TRN2 NeuronCores.

Tensor-parallel over heads: core c owns heads {2c, 2c+1} (projection dim
slice [256c, 256c+256)).  Each core computes q/k/v projections for its
heads, RoPE, full softmax attention for its 8 (batch, head) pairs, and a
partial output projection against its 256-row slice of wo; the host sums
the 8 bf16 partial outputs and adds wo_b.

Schedule: a 2-deep span pipeline.  While span i's S=QK^T matmuls and exp
(scalar engine, the per-span rate limiter) run, the PE is kept fed with
span i-1's PV matmuls, ao transposes and out-projection groups, pulled
from a work queue between S groups.  This holds across batch boundaries
(the projection phase of batch b+1 also pumps the queue), so the PE never
sees a sparse stretch and the HAM clock gate stays at full rate.

All matmuls run in bf16 with f32 PSUM accumulation; softmax runs exp in
f32->bf16 on the scalar engine with denominators accumulated via an extra
ones-column on V through the PV matmul.  RoPE runs on the vector engine
as full-128-partition ops; PV scaling (1/denominator) also on vector.
"""

import json
import sys
from collections import deque

sys.path.insert(0, "/opt/trn_rl_repo")

import ml_dtypes
import numpy as np

BF16 = ml_dtypes.bfloat16

# Problem shape (hardcoded per contract).
B, T, D = 4, 2048, 2048
H = 16
N_CORES = 8
HL = H // N_CORES  # heads per core = 2
DH = D // H  # head dim = 128
DOUT = HL * DH  # per-core projection width = 256
BT = B * T  # 8192 tokens
P = 128
NK = D // P  # 16 feature chunks
SPAN = 512
NSPAN = T // SPAN  # 4 token spans per batch
NTT = T // P  # 16 token tiles per batch
NKT2 = NTT // 2  # 8 k-tile pairs per batch


# ---------------------------------------------------------------------------
# BIR legalization: the walrus build in this container rejects instructions
# carrying more than one sync wait. Engines execute their stream in order, so
# hoisting excess waits into standalone EventSemaphore instructions directly
# before the instruction (same engine) is semantically equivalent; Tile's
# dependency graph is acyclic so this cannot deadlock.
# ---------------------------------------------------------------------------


def _legalize_waits(bir_json: bytes, max_inline: int = 1, es_capacity: int = 2):
    bir = json.loads(bir_json)
    for f in bir.get("functions", []):
        for bb in f.get("blocks", []):
            out = []
            for inst in bb.get("instructions", []):
                si = inst.get("sync_info")
                waits = (si or {}).get("on_wait") or []
                cap = (
                    es_capacity
                    if inst.get("opcode") == "EventSemaphore"
                    else max_inline
                )
                if len(waits) > cap:
                    keep, excess = waits[:cap], waits[cap:]
                    for ci in range(0, len(excess), es_capacity):
                        out.append(
                            {
                                "debug": inst.get("debug", 0),
                                "engine": inst["engine"],
                                "ins": [],
                                "name": f"{inst['name']}_xw{ci}",
                                "opcode": "EventSemaphore",
                                "outs": [],
                                "sync_info": {
                                    "on_update": [],
                                    "on_wait": excess[ci : ci + es_capacity],
                                },
                            }
                        )
                    si["on_wait"] = keep
                out.append(inst)
            bb["instructions"] = out
    return json.dumps(bir).encode()


_patched = False


def _install_compile_patch():
    global _patched
    if _patched:
        return
    _patched = True
    from concourse import bass2jax, bass_utils

    orig = bass_utils.compile_bir_kernel

    def patched_compile(bir_json, tmpdir, neff_name="file.neff"):
        return orig(_legalize_waits(bir_json), tmpdir, neff_name)

    bass2jax.compile_bir_kernel = patched_compile


# ---------------------------------------------------------------------------
# Kernel builder (one SPMD graph; per-core behavior differs only via inputs)
# ---------------------------------------------------------------------------


def _build_nc():
    import concourse.bass as bass
    import concourse.tile as tile
    from concourse import mybir
    from concourse.masks import make_identity

    f32 = mybir.dt.float32
    bf16 = mybir.dt.bfloat16

    nc = bass.Bass()
    xT = nc.declare_dram_parameter("xT", [D, BT], bf16, isOutput=False)
    wqT = nc.declare_dram_parameter("wqT", [D + 2, DOUT], bf16, isOutput=False)
    wkT = nc.declare_dram_parameter("wkT", [D + 2, DOUT], bf16, isOutput=False)
    wvT = nc.declare_dram_parameter("wvT", [D + 2, DOUT], bf16, isOutput=False)
    woT = nc.declare_dram_parameter("woT", [DOUT, D], bf16, isOutput=False)
    cosT = nc.declare_dram_parameter("cosT", [DH, T], bf16, isOutput=False)
    sinT = nc.declare_dram_parameter("sinT", [DH, T], bf16, isOutput=False)
    outp = nc.declare_dram_parameter("out", [BT, D], bf16, isOutput=True)

    HDH = DH + 1  # head slot width in v_ones (128 v cols + ones col)
    hh = DH // 2
    Copy = mybir.ActivationFunctionType.Copy
    Exp = mybir.ActivationFunctionType.Exp
    add = mybir.AluOpType.add
    mult = mybir.AluOpType.mult

    with tile.TileContext(nc) as tc:
        with (
            tc.tile_pool(name="wpool", bufs=1) as wpool,
            tc.tile_pool(name="xpool", bufs=3) as xpool,
            tc.tile_pool(name="qkT", bufs=1) as qkT,
            tc.tile_pool(name="vpool", bufs=2) as vpool,
            tc.tile_pool(name="aot", bufs=1) as aotp,
            tc.tile_pool(name="aoT", bufs=2) as aoTp,
            tc.tile_pool(name="epool", bufs=2) as epool,
            tc.tile_pool(name="misc", bufs=1) as misc,
            tc.tile_pool(name="recp", bufs=4) as recp,
            tc.tile_pool(name="obuf", bufs=3) as obuf,
            tc.tile_pool(name="psS", bufs=2, space="PSUM") as psS,
            tc.tile_pool(name="ps512", bufs=2, space="PSUM") as ps512,
            tc.tile_pool(name="pspv", bufs=2, space="PSUM") as pspv,
        ):
            # ---- persistent: weights, tables, bias columns ----
            ident = wpool.tile([P, P], bf16, tag="ident")

            def load_wT(name, dram):
                # two DMAs for the 16 k-chunks: [2048, DOUT] -> [128, 16, DOUT]
                wsb = wpool.tile([P, NK, DOUT], bf16, tag=name)
                wsrc = dram[:D, :].rearrange("(ko p) d -> p ko d", p=P)
                for c4 in range(4):
                    nc.gpsimd.dma_start(
                        out=wsb[:, c4 * 4 : (c4 + 1) * 4, :],
                        in_=wsrc[:, c4 * 4 : (c4 + 1) * 4, :],
                    )
                # biases: one DMA for all heads' normal + swapped columns
                # (layout [p, r*HL+m]: constant stride in (r m) order)
                bt = wpool.tile([DH, 2 * HL], bf16, tag=f"{name}bt")
                nc.gpsimd.dma_start(
                    out=bt,
                    in_=dram[D : D + 2, :].rearrange("r (m p) -> p (r m)", p=P),
                )
                bcols = [
                    (bt[:, m : m + 1], bt[:, HL + m : HL + m + 1])
                    for m in range(HL)
                ]
                return wsb, bcols

            wq_t, wq_bc = load_wT("wq", wqT)
            deferred = {}

            def load_rest():
                cos_sb = wpool.tile([DH, T], bf16, tag="cos")
                sin_sb = wpool.tile([DH, T], bf16, tag="sin")
                nc.gpsimd.dma_start(out=cos_sb, in_=cosT[:, :])
                nc.gpsimd.dma_start(out=sin_sb, in_=sinT[:, :])
                wk_t, wk_bc = load_wT("wk", wkT)
                wv_t, _ = load_wT("wv", wvT)
                # v bias broadcast tile [P, DOUT] from the wvT bias row
                vb_bc = wpool.tile([P, DOUT], bf16, tag="vb_bc")
                wvT_brow = wvT[D : D + 1, :]
                nc.gpsimd.dma_start(
                    out=vb_bc,
                    in_=bass.AP(
                        tensor=wvT_brow.tensor,
                        offset=wvT_brow.offset,
                        ap=[[0, P], wvT_brow.ap[-1]],
                    ),
                )
                deferred.update(
                    cos_sb=cos_sb, sin_sb=sin_sb, wk_t=wk_t, wk_bc=wk_bc,
                    wv_t=wv_t, vb_bc=vb_bc,
                )

            wo_t = []

            # ---- per-batch SBUF state (tags reused each batch) ----
            qT = [qkT.tile([P, T], bf16, tag=f"qT{m}", name=f"qT{m}") for m in range(HL)]
            kT = [qkT.tile([P, T], bf16, tag=f"kT{m}", name=f"kT{m}") for m in range(HL)]

            # -------------------------------------------------------------
            # Work queue: closures each emitting one short burst of PE work
            # (~0.3-1us).  pump(n) pops and emits up to n items.
            # -------------------------------------------------------------
            pending = deque()

            def pump(n):
                for _ in range(n):
                    if not pending:
                        return
                    pending.popleft()()

            op_alt = [0]

            def emit_outproj(b, s, tt, ds, aoT_s):
                # out[tokens tt-block, ds*512:+512] partial over this core's
                # two heads; psum -> bf16 staging -> DRAM
                t0 = b * T + s * SPAN + tt * P
                ps = ps512.tile([P, SPAN], f32, tag="p512", name="ps_op")
                for m in range(HL):
                    nc.tensor.matmul(
                        ps,
                        aoT_s[m][:, tt * P : (tt + 1) * P],
                        wo_t[m][:, ds * SPAN : (ds + 1) * SPAN],
                        start=(m == 0),
                        stop=(m == HL - 1),
                    )
                ob = obuf.tile([P, SPAN], bf16, tag="ob", name="ob")
                nc.vector.tensor_copy(out=ob, in_=ps)
                nc.sync.dma_start(
                    out=outp[t0 : t0 + P, ds * SPAN : (ds + 1) * SPAN], in_=ob
                )

            def emit_pv(m, tt, pvblk, ao_s, e_tiles, v_t):
                # PV for one 128-token q block: accumulate over all 16 k tiles
                po = pspv.tile([P, DH + 1], f32, tag="pv", name="po")
                for kt in range(NTT):
                    ek = e_tiles[m][kt // 2]
                    sl = slice(
                        (kt % 2) * SPAN + tt * P, (kt % 2) * SPAN + (tt + 1) * P
                    )
                    nc.tensor.matmul(
                        po,
                        ek[:, sl],
                        v_t[kt][:, m * HDH : (m + 1) * HDH],
                        start=(kt == 0),
                        stop=(kt == NTT - 1),
                    )
                rec = recp.tile([P, 1], f32, tag="rec", name="rec")
                nc.vector.reciprocal(rec, po[:, DH : DH + 1])
                nc.vector.tensor_scalar_mul(
                    ao_s[tt][:, m * DH : (m + 1) * DH], po[:, 0:DH], rec
                )

            def emit_transpose(m, tt, ptblk, ao_s, aoT_s):
                pt = pspv.tile([P, P], bf16, tag="pv", name="pt")
                nc.tensor.transpose(pt, ao_s[tt][:, m * DH : (m + 1) * DH], ident)
                nc.vector.tensor_copy(
                    out=aoT_s[m][:, tt * P : (tt + 1) * P], in_=pt
                )

            def enqueue_span(b, s, e_tiles, v_t):
                aoT_s = [aoTp.tile([P, SPAN], bf16, tag=f"aoT{m}", name=f"aoT{m}") for m in range(HL)]
                pvblk = None
                ptblk = None
                ao_s = [
                    aotp.tile([P, DOUT], bf16, tag=f"ao{tt}", name=f"ao{tt}")
                    for tt in range(SPAN // P)
                ]
                ntt = SPAN // P
                if b == B - 1 and s == NSPAN - 1:
                    for tt in range(ntt):
                        for m in range(HL):
                            pending.append(
                                lambda m=m, tt=tt: emit_pv(
                                    m, tt, pvblk, ao_s, e_tiles, v_t
                                )
                            )
                        for m in range(HL):
                            pending.append(
                                lambda m=m, tt=tt: emit_transpose(
                                    m, tt, ptblk, ao_s, aoT_s
                                )
                            )
                        for ds in range(D // SPAN):
                            pending.append(
                                lambda tt=tt, ds=ds: emit_outproj(
                                    b, s, tt, ds, aoT_s
                                )
                            )
                    return
                for tt in range(ntt):
                    for m in range(HL):
                        pending.append(
                            lambda m=m, tt=tt: emit_pv(m, tt, pvblk, ao_s, e_tiles, v_t)
                        )
                    if tt > 0:
                        for m in range(HL):
                            pending.append(
                                lambda m=m, tt=tt - 1: emit_transpose(
                                    m, tt, ptblk, ao_s, aoT_s
                                )
                            )
                        for ds in range(D // SPAN):
                            pending.append(
                                lambda tt=tt - 1, ds=ds: emit_outproj(
                                    b, s, tt, ds, aoT_s
                                )
                            )
                for m in range(HL):
                    pending.append(
                        lambda m=m: emit_transpose(m, ntt - 1, ptblk, ao_s, aoT_s)
                    )
                for ds in range(D // SPAN):
                    pending.append(
                        lambda ds=ds: emit_outproj(b, s, ntt - 1, ds, aoT_s)
                    )

            # -------------------------------------------------------------
            # Main emission
            # -------------------------------------------------------------
            for b in range(B):
                v_t = [
                    vpool.tile([P, HL * HDH], bf16, tag=f"v{tt}", name=f"v{tt}")
                    for tt in range(NTT)
                ]

                # ---- projection phase: QKV + RoPE, span pairs ----
                for s2 in range(NSPAN // 2):
                    xts = []
                    xsrcs = []
                    for half in range(2):
                        s = 2 * s2 + half
                        t0 = b * T + s * SPAN
                        xk = xpool.tile([P, NK, SPAN], bf16, tag="x", name="xk")
                        xsrcs.append(
                            xT[:, t0 : t0 + SPAN].rearrange("(ko p) t -> p ko t", p=P)
                        )
                        xts.append(xk)
                    # interleave halves so chunk k of both spans lands early
                    # (the k-loop consumes both halves per k)
                    for c4 in range(4):
                        for half in range(2):
                            nc.gpsimd.dma_start(
                                out=xts[half][:, c4 * 4 : (c4 + 1) * 4, :],
                                in_=xsrcs[half][:, c4 * 4 : (c4 + 1) * 4, :],
                            )
                    if not deferred:
                        load_rest()
                    cos_sb = deferred["cos_sb"]
                    sin_sb = deferred["sin_sb"]
                    wk_t = deferred["wk_t"]
                    wk_bc = deferred["wk_bc"]
                    wv_t = deferred["wv_t"]
                    vb_bc = deferred["vb_bc"]
                    sl2 = slice(2 * s2 * SPAN, (2 * s2 + 2) * SPAN)  # 1024 tokens
                    # q/k over the span pair: [128, 1024] psum, LDW shared
                    for dst, wsb, bcols in ((qT, wq_t, wq_bc), (kT, wk_t, wk_bc)):
                        for m in range(HL):
                            ps = psS.tile([P, 2 * SPAN], f32, tag="pS", name="ps_qk")
                            for k in range(NK):
                                for half in range(2):
                                    nc.tensor.matmul(
                                        ps[:, half * SPAN : (half + 1) * SPAN],
                                        wsb[:, k, m * P : (m + 1) * P],
                                        xts[half][:, k, :],
                                        start=(k == 0),
                                        stop=(k == NK - 1),
                                    )
                            # RoPE, full-partition ops; rotate-half swap is
                            # done in the PSUM-reading STTs (PSUM+SB pairs may
                            # differ in base partition; SB+SB may not), rotate
                            # sign folded into sinT ([-sin; +sin]),
                            # swapped-halves bias column:
                            #   tC       = (ps + b) * cosF
                            #   tS[0:64] = (ps[64:]+b_hi) * (-sin)
                            #   tS[64:]  = (ps[:64]+b_lo) * (+sin)
                            #   out = tC + tS
                            cs = cos_sb[:, sl2]
                            sn = sin_sb[:, sl2]
                            bc, bcs = bcols[m]
                            tC = misc.tile([P, 2 * SPAN], bf16, tag="rC", name="tC")
                            tS = misc.tile([P, 2 * SPAN], bf16, tag="rS", name="tS")
                            nc.vector.scalar_tensor_tensor(
                                tC, ps, bc, cs, add, mult
                            )
                            nc.vector.scalar_tensor_tensor(
                                tS[0:hh, :], ps[hh : 2 * hh, :], bcs[0:hh],
                                sn[0:hh, :], add, mult,
                            )
                            nc.vector.scalar_tensor_tensor(
                                tS[hh : 2 * hh, :], ps[0:hh, :], bcs[hh : 2 * hh],
                                sn[hh : 2 * hh, :], add, mult,
                            )
                            nc.vector.tensor_add(dst[m][:, sl2], tC, tS)

                    # v: per 128-token tile
                    for half in range(2):
                        s = 2 * s2 + half
                        for tt in range(SPAN // P):
                            gt = s * (SPAN // P) + tt
                            sl_p = slice(tt * P, (tt + 1) * P)
                            ps = ps512.tile([P, SPAN], f32, tag="p512", name="ps_v")
                            psv = ps[:, :DOUT]
                            for k in range(NK):
                                nc.tensor.matmul(
                                    psv,
                                    xts[half][:, k, sl_p],
                                    wv_t[:, k, :],
                                    start=(k == 0),
                                    stop=(k == NK - 1),
                                )
                            vt = v_t[gt]
                            ones_ap = bass.AP(
                                tensor=vt.tensor,
                                offset=vt.offset + DH,
                                ap=[vt.ap[0], [HDH, HL]],
                            )
                            nc.vector.memset(ones_ap, 1.0)
                            for m in range(HL):
                                nc.vector.tensor_add(
                                    vt[:, m * HDH : m * HDH + DH],
                                    psv[:, m * DH : (m + 1) * DH],
                                    vb_bc[:, m * DH : (m + 1) * DH],
                                )

                if b == 0:
                    # wo and the transpose identity are needed only from the
                    # first pumped out-proj/transpose items (during b=0
                    # attention); late position keeps them off the critical
                    # startup path.
                    make_identity(nc, ident)
                    for m in range(HL):
                        t = wpool.tile([P, D], bf16, tag=f"wo{m}")
                        nc.gpsimd.dma_start(out=t, in_=woT[m * P : (m + 1) * P, :])
                        wo_t.append(t)

                # ---- attention: S+exp per span, queue pumped between ----
                for s in range(NSPAN):
                    sl_q = slice(s * SPAN, (s + 1) * SPAN)
                    e_tiles = {m: [] for m in range(HL)}
                    for kt2 in range(NKT2):
                        for m in range(HL):
                            ps = psS.tile([P, 2 * SPAN], f32, tag="pS", name="ps_s")
                            for half in range(2):
                                nc.tensor.matmul(
                                    ps[:, half * SPAN : (half + 1) * SPAN],
                                    kT[m][
                                        :,
                                        (2 * kt2 + half) * P : (2 * kt2 + half + 1) * P,
                                    ],
                                    qT[m][:, sl_q],
                                    start=True,
                                    stop=True,
                                )
                            e = epool.tile(
                                [P, 2 * SPAN], bf16, tag=f"e{m}_{kt2}", name=f"e{m}"
                            )
                            nc.scalar.activation(out=e, in_=ps, func=Exp)
                            e_tiles[m].append(e)
                        pump(2)
                    # Drain all older-span items before enqueueing this span:
                    # keeps every reader of an e/aoT buffer version emitted
                    # before the next writer of that buffer (bufs=2 safety),
                    # and leaves exactly one span of filler in the queue.
                    pump(len(pending))
                    enqueue_span(b, s, e_tiles, v_t)

            # ---- drain remaining queued work (last span's PV/T/op) ----
            pump(len(pending))
    return nc


_nc_cache = None


def _get_nc():
    global _nc_cache
    if _nc_cache is None:
        _nc_cache = _build_nc()
    return _nc_cache


# ---------------------------------------------------------------------------
# Host wrapper
# ---------------------------------------------------------------------------


def _prep_inputs(x, pos, wq_w, wq_b, wk_w, wk_b, wv_w, wv_b, wo_w, wo_b):
    x2 = np.asarray(x, np.float32).reshape(BT, D)
    xT = np.ascontiguousarray(x2.T).astype(BF16)

    pos1 = np.asarray(pos, np.float32).reshape(T)
    freq = (1.0 / 10000.0 ** (np.arange(0, DH, 2, np.float32) / DH)).astype(np.float32)
    ang = pos1[None, :] * freq[:, None]  # [64, T]
    cos1 = np.cos(ang).astype(BF16)
    sin1 = np.sin(ang).astype(BF16)
    # duplicate across both dh halves -> [128, T]; sin carries the
    # rotate-half sign: rows 0:64 = -sin, rows 64:128 = +sin
    cosT = np.concatenate([cos1, cos1], axis=0)
    sinT = np.concatenate([-sin1, sin1], axis=0)

    scale = np.float32(1.0 / np.sqrt(DH))

    def wslice(w, bvec, c, s=None, swap_row=False):
        w = np.asarray(w, np.float32)
        bvec = np.asarray(bvec, np.float32)
        ws = w[c * DOUT : (c + 1) * DOUT]  # [256, D]
        bs = bvec[c * DOUT : (c + 1) * DOUT]
        if s is not None:
            ws = ws * s
            bs = bs * s
        rows = D + 2 if swap_row else D + 1
        out = np.empty((rows, DOUT), BF16)
        out[:D] = ws.T.astype(BF16)
        out[D] = bs.astype(BF16)
        if swap_row:
            # per-head swapped dh halves of the bias
            bsw = bs.reshape(HL, 2, DH // 2)[:, ::-1, :].reshape(DOUT)
            out[D + 1] = bsw.astype(BF16)
        return out

    in_maps = []
    for c in range(N_CORES):
        woTc = (
            np.asarray(wo_w, np.float32)[:, c * DOUT : (c + 1) * DOUT]
            .T.astype(BF16)
            .copy()
        )
        in_maps.append(
            {
                "xT": xT,
                "wqT": wslice(wq_w, wq_b, c, scale, swap_row=True),
                "wkT": wslice(wk_w, wk_b, c, swap_row=True),
                "wvT": wslice(wv_w, wv_b, c, swap_row=True),
                "woT": woTc,
                "cosT": cosT,
                "sinT": sinT,
            }
        )
    return in_maps


def _run(in_maps, trace=False):
    _install_compile_patch()
    from concourse.bass_utils import run_bass_kernel_spmd

    nc = _get_nc()
    return run_bass_kernel_spmd(
        nc, in_maps, core_ids=list(range(N_CORES)), trace=trace
    )


def kernel(**inputs):
    inputs = {k: np.asarray(v) for k, v in inputs.items()}
    in_maps = _prep_inputs(**inputs)
    r = _run(in_maps, trace=False)
    acc = np.zeros((BT, D), np.float32)
    for c in range(N_CORES):
        acc += r.results[c]["out"].astype(np.float32)
    acc += np.asarray(inputs["wo_b"], np.float32)
    return acc.reshape(B, T, D)


# revision 24
# speedup vs baseline: 1.2027x; 1.0064x over previous
"""Multi-head attention (B=4, T=2048, dim=2048, H=16, RoPE) on 8 TRN2 NeuronCores.

Tensor-parallel over heads: core c owns heads {2c, 2c+1} (projection dim
slice [256c, 256c+256)).  Each core computes q/k/v projections for its
heads, RoPE, full softmax attention for its 8 (batch, head) pairs, and a
partial output projection against its 256-row slice of wo; the host sums
the 8 bf16 partial outputs and adds wo_b.

Schedule: a 2-deep span pipeline.  While span i's S=QK^T matmuls and exp
(scalar engine, the per-span rate limiter) run, the PE is kept fed with
span i-1's PV matmuls, ao transposes and out-projection groups, pulled
from a work queue between S groups.  This holds across batch boundaries
(the projection phase of batch b+1 also pumps the queue), so the PE never
sees a sparse stretch and the HAM clock gate stays at full rate.

All matmuls run in bf16 with f32 PSUM accumulation; softmax runs exp in
f32->bf16 on the scalar engine with denominators accumulated via an extra
ones-column on V through the PV matmul.  RoPE runs on the vector engine
as full-128-partition ops; PV scaling (1/denominator) also on vector.
"""

import json
import sys
from collections import deque

sys.path.insert(0, "/opt/trn_rl_repo")

import ml_dtypes
import numpy as np

BF16 = ml_dtypes.bfloat16

# Problem shape (hardcoded per contract).
B, T, D = 4, 2048, 2048
H = 16
N_CORES = 8
HL = H // N_CORES  # heads per core = 2
DH = D // H  # head dim = 128
DOUT = HL * DH  # per-core projection width = 256
BT = B * T  # 8192 tokens
P = 128
NK = D // P  # 16 feature chunks
SPAN = 512
NSPAN = T // SPAN  # 4 token spans per batch
NTT = T // P  # 16 token tiles per batch
NKT2 = NTT // 2  # 8 k-tile pairs per batch


# ---------------------------------------------------------------------------
# BIR legalization: the walrus build in this container rejects instructions
# carrying more than one sync wait. Engines execute their stream in order, so
# hoisting excess waits into standalone EventSemaphore instructions directly
# before the instruction (same engine) is semantically equivalent; Tile's
# dependency graph is acyclic so this cannot deadlock.
# ---------------------------------------------------------------------------


def _legalize_waits(bir_json: bytes, max_inline: int = 1, es_capacity: int = 2):
    bir = json.loads(bir_json)
    for f in bir.get("functions", []):
        for bb in f.get("blocks", []):
            out = []
            for inst in bb.get("instructions", []):
                si = inst.get("sync_info")
                waits = (si or {}).get("on_wait") or []
                cap = (
                    es_capacity
                    if inst.get("opcode") == "EventSemaphore"
                    else max_inline
                )
                if len(waits) > cap:
                    keep, excess = waits[:cap], waits[cap:]
                    for ci in range(0, len(excess), es_capacity):
                        out.append(
                            {
                                "debug": inst.get("debug", 0),
                                "engine": inst["engine"],
                                "ins": [],
                                "name": f"{inst['name']}_xw{ci}",
                                "opcode": "EventSemaphore",
                                "outs": [],
                                "sync_info": {
                                    "on_update": [],
                                    "on_wait": excess[ci : ci + es_capacity],
                                },
                            }
                        )
                    si["on_wait"] = keep
                out.append(inst)
            bb["instructions"] = out
    return json.dumps(bir).encode()


_patched = False


def _install_compile_patch():
    global _patched
    if _patched:
        return
    _patched = True
    from concourse import bass2jax, bass_utils

    orig = bass_utils.compile_bir_kernel

    def patched_compile(bir_json, tmpdir, neff_name="file.neff"):
        return orig(_legalize_waits(bir_json), tmpdir, neff_name)

    bass2jax.compile_bir_kernel = patched_compile


# ---------------------------------------------------------------------------
# Kernel builder (one SPMD graph; per-core behavior differs only via inputs)
# ---------------------------------------------------------------------------


def _build_nc():
    import concourse.bass as bass
    import concourse.tile as tile
    from concourse import mybir
    from concourse.masks import make_identity

    f32 = mybir.dt.float32
    bf16 = mybir.dt.bfloat16

    nc = bass.Bass()
    xT = nc.declare_dram_parameter("xT", [D, BT], bf16, isOutput=False)
    wqT = nc.declare_dram_parameter("wqT", [D + 2, DOUT], bf16, isOutput=False)
    wkT = nc.declare_dram_parameter("wkT", [D + 2, DOUT], bf16, isOutput=False)
    wvT = nc.declare_dram_parameter("wvT", [D + 2, DOUT], bf16, isOutput=False)
    woT = nc.declare_dram_parameter("woT", [DOUT, D], bf16, isOutput=False)
    cosT = nc.declare_dram_parameter("cosT", [DH, T], bf16, isOutput=False)
    sinT = nc.declare_dram_parameter("sinT", [DH, T], bf16, isOutput=False)
    outp = nc.declare_dram_parameter("out", [BT, D], bf16, isOutput=True)

    HDH = DH + 1  # head slot width in v_ones (128 v cols + ones col)
    hh = DH // 2
    Copy = mybir.ActivationFunctionType.Copy
    Exp = mybir.ActivationFunctionType.Exp
    add = mybir.AluOpType.add
    mult = mybir.AluOpType.mult

    with tile.TileContext(nc) as tc:
        with (
            tc.tile_pool(name="wpool", bufs=1) as wpool,
            tc.tile_pool(name="xpool", bufs=3) as xpool,
            tc.tile_pool(name="qkT", bufs=1) as qkT,
            tc.tile_pool(name="vpool", bufs=2) as vpool,
            tc.tile_pool(name="aot", bufs=1) as aotp,
            tc.tile_pool(name="aoT", bufs=2) as aoTp,
            tc.tile_pool(name="epool", bufs=2) as epool,
            tc.tile_pool(name="misc", bufs=1) as misc,
            tc.tile_pool(name="recp", bufs=4) as recp,
            tc.tile_pool(name="obuf", bufs=2) as obuf,
            tc.tile_pool(name="psS", bufs=2, space="PSUM") as psS,
            tc.tile_pool(name="ps512", bufs=2, space="PSUM") as ps512,
            tc.tile_pool(name="pspv", bufs=2, space="PSUM") as pspv,
        ):
            # ---- persistent: weights, tables, bias columns ----
            ident = wpool.tile([P, P], bf16, tag="ident")

            def load_wT(name, dram):
                # two DMAs for the 16 k-chunks: [2048, DOUT] -> [128, 16, DOUT]
                wsb = wpool.tile([P, NK, DOUT], bf16, tag=name)
                wsrc = dram[:D, :].rearrange("(ko p) d -> p ko d", p=P)
                for c4 in range(4):
                    nc.gpsimd.dma_start(
                        out=wsb[:, c4 * 4 : (c4 + 1) * 4, :],
                        in_=wsrc[:, c4 * 4 : (c4 + 1) * 4, :],
                    )
                # biases: one DMA for all heads' normal + swapped columns
                # (layout [p, r*HL+m]: constant stride in (r m) order)
                bt = wpool.tile([DH, 2 * HL], bf16, tag=f"{name}bt")
                nc.gpsimd.dma_start(
                    out=bt,
                    in_=dram[D : D + 2, :].rearrange("r (m p) -> p (r m)", p=P),
                )
                bcols = [
                    (bt[:, m : m + 1], bt[:, HL + m : HL + m + 1])
                    for m in range(HL)
                ]
                return wsb, bcols

            wq_t, wq_bc = load_wT("wq", wqT)
            deferred = {}

            def load_rest():
                cos_sb = wpool.tile([DH, T], bf16, tag="cos")
                sin_sb = wpool.tile([DH, T], bf16, tag="sin")
                nc.gpsimd.dma_start(out=cos_sb, in_=cosT[:, :])
                nc.gpsimd.dma_start(out=sin_sb, in_=sinT[:, :])
                wk_t, wk_bc = load_wT("wk", wkT)
                wv_t, _ = load_wT("wv", wvT)
                # v bias broadcast tile [P, DOUT] from the wvT bias row
                vb_bc = wpool.tile([P, DOUT], bf16, tag="vb_bc")
                wvT_brow = wvT[D : D + 1, :]
                nc.gpsimd.dma_start(
                    out=vb_bc,
                    in_=bass.AP(
                        tensor=wvT_brow.tensor,
                        offset=wvT_brow.offset,
                        ap=[[0, P], wvT_brow.ap[-1]],
                    ),
                )
                deferred.update(
                    cos_sb=cos_sb, sin_sb=sin_sb, wk_t=wk_t, wk_bc=wk_bc,
                    wv_t=wv_t, vb_bc=vb_bc,
                )

            wo_t = []

            # ---- per-batch SBUF state (tags reused each batch) ----
            qT = [qkT.tile([P, T], bf16, tag=f"qT{m}", name=f"qT{m}") for m in range(HL)]
            kT = [qkT.tile([P, T], bf16, tag=f"kT{m}", name=f"kT{m}") for m in range(HL)]

            # -------------------------------------------------------------
            # Work queue: closures each emitting one short burst of PE work
            # (~0.3-1us).  pump(n) pops and emits up to n items.
            # -------------------------------------------------------------
            pending = deque()

            def pump(n):
                for _ in range(n):
                    if not pending:
                        return
                    pending.popleft()()

            op_alt = [0]

            ob_cur = [None]

            def emit_outproj(b, s, tt, ds, aoT_s):
                # out[tokens tt-block, ds*512:+512] partial over this core's
                # two heads; psum -> bf16 staging; one DMA per full token row
                t0 = b * T + s * SPAN + tt * P
                ps = ps512.tile([P, SPAN], f32, tag="p512", name="ps_op")
                for m in range(HL):
                    nc.tensor.matmul(
                        ps,
                        aoT_s[m][:, tt * P : (tt + 1) * P],
                        wo_t[m][:, ds * SPAN : (ds + 1) * SPAN],
                        start=(m == 0),
                        stop=(m == HL - 1),
                    )
                if ds == 0:
                    ob_cur[0] = obuf.tile([P, D], bf16, tag="ob", name="ob")
                ob = ob_cur[0]
                op_alt[0] = (op_alt[0] + 1) % 4
                dst = ob[:, ds * SPAN : (ds + 1) * SPAN]
                if op_alt[0] == 0:
                    nc.scalar.activation(out=dst, in_=ps, func=Copy)
                else:
                    nc.vector.tensor_copy(out=dst, in_=ps)
                if ds == D // SPAN - 1:
                    nc.sync.dma_start(out=outp[t0 : t0 + P, :], in_=ob)

            def emit_pv(m, tt, pvblk, ao_s, e_tiles, v_t):
                # PV for one 128-token q block: accumulate over all 16 k tiles
                po = pspv.tile([P, DH + 1], f32, tag="pv", name="po")
                for kt in range(NTT):
                    ek = e_tiles[m][kt // 2]
                    sl = slice(
                        (kt % 2) * SPAN + tt * P, (kt % 2) * SPAN + (tt + 1) * P
                    )
                    nc.tensor.matmul(
                        po,
                        ek[:, sl],
                        v_t[kt][:, m * HDH : (m + 1) * HDH],
                        start=(kt == 0),
                        stop=(kt == NTT - 1),
                    )
                rec = recp.tile([P, 1], f32, tag="rec", name="rec")
                nc.vector.reciprocal(rec, po[:, DH : DH + 1])
                nc.vector.tensor_scalar_mul(
                    ao_s[tt][:, m * DH : (m + 1) * DH], po[:, 0:DH], rec
                )

            def emit_transpose(m, tt, ptblk, ao_s, aoT_s):
                pt = pspv.tile([P, P], bf16, tag="pv", name="pt")
                nc.tensor.transpose(pt, ao_s[tt][:, m * DH : (m + 1) * DH], ident)
                nc.vector.tensor_copy(
                    out=aoT_s[m][:, tt * P : (tt + 1) * P], in_=pt
                )

            def enqueue_span(b, s, e_tiles, v_t):
                aoT_s = [aoTp.tile([P, SPAN], bf16, tag=f"aoT{m}", name=f"aoT{m}") for m in range(HL)]
                pvblk = None
                ptblk = None
                ao_s = [
                    aotp.tile([P, DOUT], bf16, tag=f"ao{tt}", name=f"ao{tt}")
                    for tt in range(SPAN // P)
                ]
                ntt = SPAN // P
                if b == B - 1 and s == NSPAN - 1:
                    for tt in range(ntt):
                        for m in range(HL):
                            pending.append(
                                lambda m=m, tt=tt: emit_pv(
                                    m, tt, pvblk, ao_s, e_tiles, v_t
                                )
                            )
                        for m in range(HL):
                            pending.append(
                                lambda m=m, tt=tt: emit_transpose(
                                    m, tt, ptblk, ao_s, aoT_s
                                )
                            )
                        for ds in range(D // SPAN):
                            pending.append(
                                lambda tt=tt, ds=ds: emit_outproj(
                                    b, s, tt, ds, aoT_s
                                )
                            )
                    return
                for tt in range(ntt):
                    for m in range(HL):
                        pending.append(
                            lambda m=m, tt=tt: emit_pv(m, tt, pvblk, ao_s, e_tiles, v_t)
                        )
                    if tt > 0:
                        for m in range(HL):
                            pending.append(
                                lambda m=m, tt=tt - 1: emit_transpose(
                                    m, tt, ptblk, ao_s, aoT_s
                                )
                            )
                        for ds in range(D // SPAN):
                            pending.append(
                                lambda tt=tt - 1, ds=ds: emit_outproj(
                                    b, s, tt, ds, aoT_s
                                )
                            )
                for m in range(HL):
                    pending.append(
                        lambda m=m: emit_transpose(m, ntt - 1, ptblk, ao_s, aoT_s)
                    )
                for ds in range(D // SPAN):
                    pending.append(
                        lambda ds=ds: emit_outproj(b, s, ntt - 1, ds, aoT_s)
                    )

            # -------------------------------------------------------------
            # Main emission
            # -------------------------------------------------------------
            for b in range(B):
                v_t = [
                    vpool.tile([P, HL * HDH], bf16, tag=f"v{tt}", name=f"v{tt}")
                    for tt in range(NTT)
                ]

                # ---- projection phase: QKV + RoPE, span pairs ----
                for s2 in range(NSPAN // 2):
                    xts = []
                    xsrcs = []
                    for half in range(2):
                        s = 2 * s2 + half
                        t0 = b * T + s * SPAN
                        xk = xpool.tile([P, NK, SPAN], bf16, tag="x", name="xk")
                        xsrcs.append(
                            xT[:, t0 : t0 + SPAN].rearrange("(ko p) t -> p ko t", p=P)
                        )
                        xts.append(xk)
                    # interleave halves so chunk k of both spans lands early
                    # (the k-loop consumes both halves per k); finer leading
                    # chunks on the very first load so the first matmul can
                    # start sooner
                    plan = (
                        [(0, 2), (2, 2), (4, 4), (8, 4), (12, 4)]
                        if (b == 0 and s2 == 0)
                        else [(0, 4), (4, 4), (8, 4), (12, 4)]
                    )
                    for k0, kn in plan:
                        for half in range(2):
                            nc.gpsimd.dma_start(
                                out=xts[half][:, k0 : k0 + kn, :],
                                in_=xsrcs[half][:, k0 : k0 + kn, :],
                            )
                    if not deferred:
                        load_rest()
                    cos_sb = deferred["cos_sb"]
                    sin_sb = deferred["sin_sb"]
                    wk_t = deferred["wk_t"]
                    wk_bc = deferred["wk_bc"]
                    wv_t = deferred["wv_t"]
                    vb_bc = deferred["vb_bc"]
                    sl2 = slice(2 * s2 * SPAN, (2 * s2 + 2) * SPAN)  # 1024 tokens
                    # q/k over the span pair: [128, 1024] psum, LDW shared
                    for dst, wsb, bcols in ((qT, wq_t, wq_bc), (kT, wk_t, wk_bc)):
                        for m in range(HL):
                            ps = psS.tile([P, 2 * SPAN], f32, tag="pS", name="ps_qk")
                            for k in range(NK):
                                for half in range(2):
                                    nc.tensor.matmul(
                                        ps[:, half * SPAN : (half + 1) * SPAN],
                                        wsb[:, k, m * P : (m + 1) * P],
                                        xts[half][:, k, :],
                                        start=(k == 0),
                                        stop=(k == NK - 1),
                                    )
                            # RoPE, full-partition ops; rotate-half swap is
                            # done in the PSUM-reading STTs (PSUM+SB pairs may
                            # differ in base partition; SB+SB may not), rotate
                            # sign folded into sinT ([-sin; +sin]),
                            # swapped-halves bias column:
                            #   tC       = (ps + b) * cosF
                            #   tS[0:64] = (ps[64:]+b_hi) * (-sin)
                            #   tS[64:]  = (ps[:64]+b_lo) * (+sin)
                            #   out = tC + tS
                            cs = cos_sb[:, sl2]
                            sn = sin_sb[:, sl2]
                            bc, bcs = bcols[m]
                            tC = misc.tile([P, 2 * SPAN], bf16, tag="rC", name="tC")
                            tS = misc.tile([P, 2 * SPAN], bf16, tag="rS", name="tS")
                            nc.vector.scalar_tensor_tensor(
                                tC, ps, bc, cs, add, mult
                            )
                            nc.vector.scalar_tensor_tensor(
                                tS[0:hh, :], ps[hh : 2 * hh, :], bcs[0:hh],
                                sn[0:hh, :], add, mult,
                            )
                            nc.vector.scalar_tensor_tensor(
                                tS[hh : 2 * hh, :], ps[0:hh, :], bcs[hh : 2 * hh],
                                sn[hh : 2 * hh, :], add, mult,
                            )
                            nc.vector.tensor_add(dst[m][:, sl2], tC, tS)

                    # v: per 128-token tile
                    for half in range(2):
                        s = 2 * s2 + half
                        for tt in range(SPAN // P):
                            gt = s * (SPAN // P) + tt
                            sl_p = slice(tt * P, (tt + 1) * P)
                            ps = ps512.tile([P, SPAN], f32, tag="p512", name="ps_v")
                            psv = ps[:, :DOUT]
                            for k in range(NK):
                                nc.tensor.matmul(
                                    psv,
                                    xts[half][:, k, sl_p],
                                    wv_t[:, k, :],
                                    start=(k == 0),
                                    stop=(k == NK - 1),
                                )
                            vt = v_t[gt]
                            ones_ap = bass.AP(
                                tensor=vt.tensor,
                                offset=vt.offset + DH,
                                ap=[vt.ap[0], [HDH, HL]],
                            )
                            nc.vector.memset(ones_ap, 1.0)
                            for m in range(HL):
                                nc.vector.tensor_add(
                                    vt[:, m * HDH : m * HDH + DH],
                                    psv[:, m * DH : (m + 1) * DH],
                                    vb_bc[:, m * DH : (m + 1) * DH],
                                )

                if b == 0:
                    # wo and the transpose identity are needed only from the
                    # first pumped out-proj/transpose items (during b=0
                    # attention); late position keeps them off the critical
                    # startup path.
                    make_identity(nc, ident)
                    for m in range(HL):
                        t = wpool.tile([P, D], bf16, tag=f"wo{m}")
                        nc.gpsimd.dma_start(out=t, in_=woT[m * P : (m + 1) * P, :])
                        wo_t.append(t)

                # ---- attention: S+exp per span, queue pumped between ----
                for s in range(NSPAN):
                    sl_q = slice(s * SPAN, (s + 1) * SPAN)
                    e_tiles = {m: [] for m in range(HL)}
                    for kt2 in range(NKT2):
                        for m in range(HL):
                            ps = psS.tile([P, 2 * SPAN], f32, tag="pS", name="ps_s")
                            for half in range(2):
                                nc.tensor.matmul(
                                    ps[:, half * SPAN : (half + 1) * SPAN],
                                    kT[m][
                                        :,
                                        (2 * kt2 + half) * P : (2 * kt2 + half + 1) * P,
                                    ],
                                    qT[m][:, sl_q],
                                    start=True,
                                    stop=True,
                                )
                            e = epool.tile(
                                [P, 2 * SPAN], bf16, tag=f"e{m}_{kt2}", name=f"e{m}"
                            )
                            nc.scalar.activation(out=e, in_=ps, func=Exp)
                            e_tiles[m].append(e)
                        pump(2)
                    # Drain all older-span items before enqueueing this span:
                    # keeps every reader of an e/aoT buffer version emitted
                    # before the next writer of that buffer (bufs=2 safety),
                    # and leaves exactly one span of filler in the queue.
                    pump(len(pending))
                    enqueue_span(b, s, e_tiles, v_t)

            # ---- drain remaining queued work (last span's PV/T/op) ----
            pump(len(pending))
    return nc


_nc_cache = None


def _get_nc():
    global _nc_cache
    if _nc_cache is None:
        _nc_cache = _build_nc()
    return _nc_cache


# ---------------------------------------------------------------------------
# Host wrapper
# ---------------------------------------------------------------------------


def _prep_inputs(x, pos, wq_w, wq_b, wk_w, wk_b, wv_w, wv_b, wo_w, wo_b):
    x2 = np.asarray(x, np.float32).reshape(BT, D)
    xT = np.ascontiguousarray(x2.T).astype(BF16)

    pos1 = np.asarray(pos, np.float32).reshape(T)
    freq = (1.0 / 10000.0 ** (np.arange(0, DH, 2, np.float32) / DH)).astype(np.float32)
    ang = pos1[None, :] * freq[:, None]  # [64, T]
    cos1 = np.cos(ang).astype(BF16)
    sin1 = np.sin(ang).astype(BF16)
    # duplicate across both dh halves -> [128, T]; sin carries the
    # rotate-half sign: rows 0:64 = -sin, rows 64:128 = +sin
    cosT = np.concatenate([cos1, cos1], axis=0)
    sinT = np.concatenate([-sin1, sin1], axis=0)

    scale = np.float32(1.0 / np.sqrt(DH))

    def wslice(w, bvec, c, s=None, swap_row=False):
        w = np.asarray(w, np.float32)
        bvec = np.asarray(bvec, np.float32)
        ws = w[c * DOUT : (c + 1) * DOUT]  # [256, D]
        bs = bvec[c * DOUT : (c + 1) * DOUT]
        if s is not None:
            ws = ws * s
            bs = bs * s
        rows = D + 2 if swap_row else D + 1
        out = np.empty((rows, DOUT), BF16)
        out[:D] = ws.T.astype(BF16)
        out[D] = bs.astype(BF16)
        if swap_row:
            # per-head swapped dh halves of the bias
            bsw = bs.reshape(HL, 2, DH // 2)[:, ::-1, :].reshape(DOUT)
            out[D + 1] = bsw.astype(BF16)
        return out

    in_maps = []
    for c in range(N_CORES):
        woTc = (
            np.asarray(wo_w, np.float32)[:, c * DOUT : (c + 1) * DOUT]
            .T.astype(BF16)
            .copy()
        )
        in_maps.append(
            {
                "xT": xT,
                "wqT": wslice(wq_w, wq_b, c, scale, swap_row=True),
                "wkT": wslice(wk_w, wk_b, c, swap_row=True),
                "wvT": wslice(wv_w, wv_b, c, swap_row=True),
                "woT": woTc,
                "cosT": cosT,
                "sinT": sinT,
            }
        )
    return in_maps


def _run(in_maps, trace=False):
    _install_compile_patch()
    from concourse.bass_utils import run_bass_kernel_spmd

    nc = _get_nc()
    return run_bass_kernel_spmd(
        nc, in_maps, core_ids=list(range(N_CORES)), trace=trace
    )


def kernel(**inputs):
    inputs = {k: np.asarray(v) for k, v in inputs.items()}
    in_maps = _prep_inputs(**inputs)
    r = _run(in_maps, trace=False)
    acc = np.zeros((BT, D), np.float32)
    for c in range(N_CORES):
        acc += r.results[c]["out"].astype(np.float32)
    acc += np.asarray(inputs["wo_b"], np.float32)
    return acc.reshape(B, T, D)


# revision 25
# speedup vs baseline: 1.2243x; 1.0180x over previous
"""Multi-head attention (B=4, T=2048, dim=2048, H=16, RoPE) on 8 TRN2 NeuronCores.

Tensor-parallel over heads: core c owns heads {2c, 2c+1} (projection dim
slice [256c, 256c+256)).  Each core computes q/k/v projections for its
heads, RoPE, full softmax attention for its 8 (batch, head) pairs, and a
partial output projection against its 256-row slice of wo; the host sums
the 8 bf16 partial outputs and adds wo_b.

Schedule: a 2-deep span pipeline.  While span i's S=QK^T matmuls and exp
(scalar engine, the per-span rate limiter) run, the PE is kept fed with
span i-1's PV matmuls, ao transposes and out-projection groups, pulled
from a work queue between S groups.  This holds across batch boundaries
(the projection phase of batch b+1 also pumps the queue), so the PE never
sees a sparse stretch and the HAM clock gate stays at full rate.

All matmuls run in bf16 with f32 PSUM accumulation; softmax runs exp in
f32->bf16 on the scalar engine with denominators accumulated via an extra
ones-column on V through the PV matmul.  RoPE runs on the vector engine
as full-128-partition ops; PV scaling (1/denominator) also on vector.
"""

import json
import sys
from collections import deque

sys.path.insert(0, "/opt/trn_rl_repo")

import ml_dtypes
import numpy as np

BF16 = ml_dtypes.bfloat16

# Problem shape (hardcoded per contract).
B, T, D = 4, 2048, 2048
H = 16
N_CORES = 8
HL = H // N_CORES  # heads per core = 2
DH = D // H  # head dim = 128
DOUT = HL * DH  # per-core projection width = 256
BT = B * T  # 8192 tokens
P = 128
NK = D // P  # 16 feature chunks
SPAN = 512
NSPAN = T // SPAN  # 4 token spans per batch
NTT = T // P  # 16 token tiles per batch
NKT2 = NTT // 2  # 8 k-tile pairs per batch


# ---------------------------------------------------------------------------
# BIR legalization: the walrus build in this container rejects instructions
# carrying more than one sync wait. Engines execute their stream in order, so
# hoisting excess waits into standalone EventSemaphore instructions directly
# before the instruction (same engine) is semantically equivalent; Tile's
# dependency graph is acyclic so this cannot deadlock.
# ---------------------------------------------------------------------------


def _legalize_waits(bir_json: bytes, max_inline: int = 1, es_capacity: int = 2):
    bir = json.loads(bir_json)
    for f in bir.get("functions", []):
        for bb in f.get("blocks", []):
            out = []
            for inst in bb.get("instructions", []):
                si = inst.get("sync_info")
                waits = (si or {}).get("on_wait") or []
                cap = (
                    es_capacity
                    if inst.get("opcode") == "EventSemaphore"
                    else max_inline
                )
                if len(waits) > cap:
                    keep, excess = waits[:cap], waits[cap:]
                    for ci in range(0, len(excess), es_capacity):
                        out.append(
                            {
                                "debug": inst.get("debug", 0),
                                "engine": inst["engine"],
                                "ins": [],
                                "name": f"{inst['name']}_xw{ci}",
                                "opcode": "EventSemaphore",
                                "outs": [],
                                "sync_info": {
                                    "on_update": [],
                                    "on_wait": excess[ci : ci + es_capacity],
                                },
                            }
                        )
                    si["on_wait"] = keep
                out.append(inst)
            bb["instructions"] = out
    return json.dumps(bir).encode()


_patched = False


def _install_compile_patch():
    global _patched
    if _patched:
        return
    _patched = True
    from concourse import bass2jax, bass_utils

    orig = bass_utils.compile_bir_kernel

    def patched_compile(bir_json, tmpdir, neff_name="file.neff"):
        return orig(_legalize_waits(bir_json), tmpdir, neff_name)

    bass2jax.compile_bir_kernel = patched_compile


# ---------------------------------------------------------------------------
# Kernel builder (one SPMD graph; per-core behavior differs only via inputs)
# ---------------------------------------------------------------------------


def _build_nc():
    import concourse.bass as bass
    import concourse.tile as tile
    from concourse import mybir
    from concourse.masks import make_identity

    f32 = mybir.dt.float32
    bf16 = mybir.dt.bfloat16

    nc = bass.Bass()
    xT = nc.declare_dram_parameter("xT", [D, BT], bf16, isOutput=False)
    wqT = nc.declare_dram_parameter("wqT", [D + 2, DOUT], bf16, isOutput=False)
    wkT = nc.declare_dram_parameter("wkT", [D + 2, DOUT], bf16, isOutput=False)
    wvT = nc.declare_dram_parameter("wvT", [D + 2, DOUT], bf16, isOutput=False)
    woT = nc.declare_dram_parameter("woT", [DOUT, D], bf16, isOutput=False)
    cosT = nc.declare_dram_parameter("cosT", [DH, T], bf16, isOutput=False)
    sinT = nc.declare_dram_parameter("sinT", [DH, T], bf16, isOutput=False)
    outp = nc.declare_dram_parameter("out", [BT, D], bf16, isOutput=True)

    HDH = DH + 1  # head slot width in v_ones (128 v cols + ones col)
    hh = DH // 2
    Copy = mybir.ActivationFunctionType.Copy
    Exp = mybir.ActivationFunctionType.Exp
    add = mybir.AluOpType.add
    mult = mybir.AluOpType.mult

    with tile.TileContext(nc) as tc:
        with (
            tc.tile_pool(name="wpool", bufs=1) as wpool,
            tc.tile_pool(name="xpool", bufs=3) as xpool,
            tc.tile_pool(name="qkT", bufs=1) as qkT,
            tc.tile_pool(name="vpool", bufs=2) as vpool,
            tc.tile_pool(name="aot", bufs=1) as aotp,
            tc.tile_pool(name="aoT", bufs=2) as aoTp,
            tc.tile_pool(name="epool", bufs=2) as epool,
            tc.tile_pool(name="misc", bufs=1) as misc,
            tc.tile_pool(name="recp", bufs=4) as recp,
            tc.tile_pool(name="obuf", bufs=2) as obuf,
            tc.tile_pool(name="psS", bufs=2, space="PSUM") as psS,
            tc.tile_pool(name="ps512", bufs=2, space="PSUM") as ps512,
            tc.tile_pool(name="pspv", bufs=2, space="PSUM") as pspv,
        ):
            # ---- persistent: weights, tables, bias columns ----
            ident = wpool.tile([P, P], bf16, tag="ident")

            def load_wT(name, dram):
                # two DMAs for the 16 k-chunks: [2048, DOUT] -> [128, 16, DOUT]
                wsb = wpool.tile([P, NK, DOUT], bf16, tag=name)
                wsrc = dram[:D, :].rearrange("(ko p) d -> p ko d", p=P)
                for c4 in range(4):
                    nc.gpsimd.dma_start(
                        out=wsb[:, c4 * 4 : (c4 + 1) * 4, :],
                        in_=wsrc[:, c4 * 4 : (c4 + 1) * 4, :],
                    )
                # biases: one DMA for all heads' normal + swapped columns
                # (layout [p, r*HL+m]: constant stride in (r m) order)
                bt = wpool.tile([DH, 2 * HL], bf16, tag=f"{name}bt")
                nc.gpsimd.dma_start(
                    out=bt,
                    in_=dram[D : D + 2, :].rearrange("r (m p) -> p (r m)", p=P),
                )
                bcols = [
                    (bt[:, m : m + 1], bt[:, HL + m : HL + m + 1])
                    for m in range(HL)
                ]
                return wsb, bcols

            wq_t, wq_bc = load_wT("wq", wqT)
            deferred = {}

            def load_rest():
                cos_sb = wpool.tile([DH, T], bf16, tag="cos")
                sin_sb = wpool.tile([DH, T], bf16, tag="sin")
                nc.gpsimd.dma_start(out=cos_sb, in_=cosT[:, :])
                nc.gpsimd.dma_start(out=sin_sb, in_=sinT[:, :])
                wk_t, wk_bc = load_wT("wk", wkT)
                wv_t, _ = load_wT("wv", wvT)
                # v bias broadcast tile [P, DOUT] from the wvT bias row
                vb_bc = wpool.tile([P, DOUT], bf16, tag="vb_bc")
                wvT_brow = wvT[D : D + 1, :]
                nc.gpsimd.dma_start(
                    out=vb_bc,
                    in_=bass.AP(
                        tensor=wvT_brow.tensor,
                        offset=wvT_brow.offset,
                        ap=[[0, P], wvT_brow.ap[-1]],
                    ),
                )
                deferred.update(
                    cos_sb=cos_sb, sin_sb=sin_sb, wk_t=wk_t, wk_bc=wk_bc,
                    wv_t=wv_t, vb_bc=vb_bc,
                )

            wo_t = []

            # ---- per-batch SBUF state (tags reused each batch) ----
            qT = [qkT.tile([P, T], bf16, tag=f"qT{m}", name=f"qT{m}") for m in range(HL)]
            kT = [qkT.tile([P, T], bf16, tag=f"kT{m}", name=f"kT{m}") for m in range(HL)]

            # -------------------------------------------------------------
            # Work queue: closures each emitting one short burst of PE work
            # (~0.3-1us).  pump(n) pops and emits up to n items.
            # -------------------------------------------------------------
            pending = deque()

            def pump(n):
                for _ in range(n):
                    if not pending:
                        return
                    pending.popleft()()

            op_alt = [0]

            ob_cur = [None]

            def emit_outproj(b, s, tt, ds, aoT_s):
                # out[tokens tt-block, ds*512:+512] partial over this core's
                # two heads; psum -> bf16 staging; one DMA per full token row
                t0 = b * T + s * SPAN + tt * P
                ps = ps512.tile([P, SPAN], f32, tag="p512", name="ps_op")
                for m in range(HL):
                    nc.tensor.matmul(
                        ps,
                        aoT_s[m][:, tt * P : (tt + 1) * P],
                        wo_t[m][:, ds * SPAN : (ds + 1) * SPAN],
                        start=(m == 0),
                        stop=(m == HL - 1),
                    )
                if ds == 0:
                    ob_cur[0] = obuf.tile([P, D], bf16, tag="ob", name="ob")
                ob = ob_cur[0]
                op_alt[0] = (op_alt[0] + 1) % 4
                dst = ob[:, ds * SPAN : (ds + 1) * SPAN]
                if op_alt[0] == 0:
                    nc.scalar.activation(out=dst, in_=ps, func=Copy)
                else:
                    nc.vector.tensor_copy(out=dst, in_=ps)
                if ds == D // SPAN - 1:
                    nc.sync.dma_start(out=outp[t0 : t0 + P, :], in_=ob)

            def emit_pv(m, tt, pvblk, ao_s, e_tiles, v_t):
                # PV for one 128-token q block: accumulate over all 16 k tiles
                po = pspv.tile([P, DH + 1], f32, tag="pv", name="po")
                for kt in range(NTT):
                    ek = e_tiles[m][kt // 2]
                    sl = slice(
                        (kt % 2) * SPAN + tt * P, (kt % 2) * SPAN + (tt + 1) * P
                    )
                    nc.tensor.matmul(
                        po,
                        ek[:, sl],
                        v_t[kt][:, m * HDH : (m + 1) * HDH],
                        start=(kt == 0),
                        stop=(kt == NTT - 1),
                    )
                rec = recp.tile([P, 1], f32, tag="rec", name="rec")
                nc.vector.reciprocal(rec, po[:, DH : DH + 1])
                nc.vector.tensor_scalar_mul(
                    ao_s[tt][:, m * DH : (m + 1) * DH], po[:, 0:DH], rec
                )

            def emit_transpose(m, tt, ptblk, ao_s, aoT_s):
                pt = pspv.tile([P, P], bf16, tag="pv", name="pt")
                nc.tensor.transpose(pt, ao_s[tt][:, m * DH : (m + 1) * DH], ident)
                nc.vector.tensor_copy(
                    out=aoT_s[m][:, tt * P : (tt + 1) * P], in_=pt
                )

            def enqueue_span(b, s, e_tiles, v_t):
                aoT_s = [aoTp.tile([P, SPAN], bf16, tag=f"aoT{m}", name=f"aoT{m}") for m in range(HL)]
                pvblk = None
                ptblk = None
                ao_s = [
                    aotp.tile([P, DOUT], bf16, tag=f"ao{tt}", name=f"ao{tt}")
                    for tt in range(SPAN // P)
                ]
                ntt = SPAN // P
                if b == B - 1 and s == NSPAN - 1:
                    for tt in range(ntt):
                        for m in range(HL):
                            pending.append(
                                lambda m=m, tt=tt: emit_pv(
                                    m, tt, pvblk, ao_s, e_tiles, v_t
                                )
                            )
                        for m in range(HL):
                            pending.append(
                                lambda m=m, tt=tt: emit_transpose(
                                    m, tt, ptblk, ao_s, aoT_s
                                )
                            )
                        for ds in range(D // SPAN):
                            pending.append(
                                lambda tt=tt, ds=ds: emit_outproj(
                                    b, s, tt, ds, aoT_s
                                )
                            )
                    return
                for tt in range(ntt):
                    for m in range(HL):
                        pending.append(
                            lambda m=m, tt=tt: emit_pv(m, tt, pvblk, ao_s, e_tiles, v_t)
                        )
                    if tt > 0:
                        for m in range(HL):
                            pending.append(
                                lambda m=m, tt=tt - 1: emit_transpose(
                                    m, tt, ptblk, ao_s, aoT_s
                                )
                            )
                        for ds in range(D // SPAN):
                            pending.append(
                                lambda tt=tt - 1, ds=ds: emit_outproj(
                                    b, s, tt, ds, aoT_s
                                )
                            )
                for m in range(HL):
                    pending.append(
                        lambda m=m: emit_transpose(m, ntt - 1, ptblk, ao_s, aoT_s)
                    )
                for ds in range(D // SPAN):
                    pending.append(
                        lambda ds=ds: emit_outproj(b, s, ntt - 1, ds, aoT_s)
                    )

            # -------------------------------------------------------------
            # Main emission
            # -------------------------------------------------------------
            for b in range(B):
                v_t = [
                    vpool.tile([P, HL * HDH], bf16, tag=f"v{tt}", name=f"v{tt}")
                    for tt in range(NTT)
                ]

                # ---- projection phase: QKV + RoPE, span pairs ----
                for s2 in range(NSPAN // 2):
                    xts = []
                    xsrcs = []
                    for half in range(2):
                        s = 2 * s2 + half
                        t0 = b * T + s * SPAN
                        xk = xpool.tile([P, NK, SPAN], bf16, tag="x", name="xk")
                        xsrcs.append(
                            xT[:, t0 : t0 + SPAN].rearrange("(ko p) t -> p ko t", p=P)
                        )
                        xts.append(xk)
                    # interleave halves so chunk k of both spans lands early
                    # (the k-loop consumes both halves per k); finer leading
                    # chunks on the very first load so the first matmul can
                    # start sooner
                    plan = (
                        [(0, 2), (2, 2), (4, 4), (8, 4), (12, 4)]
                        if (b == 0 and s2 == 0)
                        else [(0, 4), (4, 4), (8, 4), (12, 4)]
                    )
                    for k0, kn in plan:
                        for half in range(2):
                            nc.gpsimd.dma_start(
                                out=xts[half][:, k0 : k0 + kn, :],
                                in_=xsrcs[half][:, k0 : k0 + kn, :],
                            )
                    if not deferred:
                        load_rest()
                    cos_sb = deferred["cos_sb"]
                    sin_sb = deferred["sin_sb"]
                    wk_t = deferred["wk_t"]
                    wk_bc = deferred["wk_bc"]
                    wv_t = deferred["wv_t"]
                    vb_bc = deferred["vb_bc"]
                    sl2 = slice(2 * s2 * SPAN, (2 * s2 + 2) * SPAN)  # 1024 tokens
                    # q/k over the span pair: [128, 1024] psum, LDW shared
                    for dst, wsb, bcols in ((qT, wq_t, wq_bc), (kT, wk_t, wk_bc)):
                        for m in range(HL):
                            ps = psS.tile([P, 2 * SPAN], f32, tag="pS", name="ps_qk")
                            for k in range(NK):
                                for half in range(2):
                                    nc.tensor.matmul(
                                        ps[:, half * SPAN : (half + 1) * SPAN],
                                        wsb[:, k, m * P : (m + 1) * P],
                                        xts[half][:, k, :],
                                        start=(k == 0),
                                        stop=(k == NK - 1),
                                    )
                            # RoPE, full-partition ops; rotate-half swap is
                            # done in the PSUM-reading STTs (PSUM+SB pairs may
                            # differ in base partition; SB+SB may not), rotate
                            # sign folded into sinT ([-sin; +sin]),
                            # swapped-halves bias column:
                            #   tC       = (ps + b) * cosF
                            #   tS[0:64] = (ps[64:]+b_hi) * (-sin)
                            #   tS[64:]  = (ps[:64]+b_lo) * (+sin)
                            #   out = tC + tS
                            cs = cos_sb[:, sl2]
                            sn = sin_sb[:, sl2]
                            bc, bcs = bcols[m]
                            tC = misc.tile([P, 2 * SPAN], bf16, tag="rC", name="tC")
                            tS = misc.tile([P, 2 * SPAN], bf16, tag="rS", name="tS")
                            nc.vector.scalar_tensor_tensor(
                                tC, ps, bc, cs, add, mult
                            )
                            nc.vector.scalar_tensor_tensor(
                                tS[0:hh, :], ps[hh : 2 * hh, :], bcs[0:hh],
                                sn[0:hh, :], add, mult,
                            )
                            nc.vector.scalar_tensor_tensor(
                                tS[hh : 2 * hh, :], ps[0:hh, :], bcs[hh : 2 * hh],
                                sn[hh : 2 * hh, :], add, mult,
                            )
                            nc.vector.tensor_add(dst[m][:, sl2], tC, tS)

                    # v: per 128-token tile
                    for half in range(2):
                        s = 2 * s2 + half
                        for tt in range(SPAN // P):
                            gt = s * (SPAN // P) + tt
                            sl_p = slice(tt * P, (tt + 1) * P)
                            ps = ps512.tile([P, SPAN], f32, tag="p512", name="ps_v")
                            psv = ps[:, :DOUT]
                            for k in range(NK):
                                nc.tensor.matmul(
                                    psv,
                                    xts[half][:, k, sl_p],
                                    wv_t[:, k, :],
                                    start=(k == 0),
                                    stop=(k == NK - 1),
                                )
                            vt = v_t[gt]
                            ones_ap = bass.AP(
                                tensor=vt.tensor,
                                offset=vt.offset + DH,
                                ap=[vt.ap[0], [HDH, HL]],
                            )
                            nc.vector.memset(ones_ap, 1.0)
                            for m in range(HL):
                                nc.vector.tensor_add(
                                    vt[:, m * HDH : m * HDH + DH],
                                    psv[:, m * DH : (m + 1) * DH],
                                    vb_bc[:, m * DH : (m + 1) * DH],
                                )

                if b == 0:
                    # wo and the transpose identity are needed only from the
                    # first pumped out-proj/transpose items (during b=0
                    # attention); late position keeps them off the critical
                    # startup path.
                    make_identity(nc, ident)
                    for m in range(HL):
                        t = wpool.tile([P, D], bf16, tag=f"wo{m}")
                        nc.gpsimd.dma_start(out=t, in_=woT[m * P : (m + 1) * P, :])
                        wo_t.append(t)

                # ---- attention: S+exp per span, queue pumped between ----
                for s in range(NSPAN):
                    sl_q = slice(s * SPAN, (s + 1) * SPAN)
                    e_tiles = {m: [] for m in range(HL)}
                    for kt2 in range(NKT2):
                        for m in range(HL):
                            ps = psS.tile([P, 2 * SPAN], f32, tag="pS", name="ps_s")
                            for half in range(2):
                                nc.tensor.matmul(
                                    ps[:, half * SPAN : (half + 1) * SPAN],
                                    kT[m][
                                        :,
                                        (2 * kt2 + half) * P : (2 * kt2 + half + 1) * P,
                                    ],
                                    qT[m][:, sl_q],
                                    start=True,
                                    stop=True,
                                )
                            e = epool.tile(
                                [P, 2 * SPAN], bf16, tag=f"e{m}_{kt2}", name=f"e{m}"
                            )
                            nc.scalar.activation(out=e, in_=ps, func=Exp)
                            e_tiles[m].append(e)
                            pump(1)
                    # Drain all older-span items before enqueueing this span:
                    # keeps every reader of an e/aoT buffer version emitted
                    # before the next writer of that buffer (bufs=2 safety),
                    # and leaves exactly one span of filler in the queue.
                    pump(len(pending))
                    enqueue_span(b, s, e_tiles, v_t)

            # ---- drain remaining queued work (last span's PV/T/op) ----
            pump(len(pending))
    return nc


_nc_cache = None


def _get_nc():
    global _nc_cache
    if _nc_cache is None:
        _nc_cache = _build_nc()
    return _nc_cache


# ---------------------------------------------------------------------------
# Host wrapper
# ---------------------------------------------------------------------------


def _prep_inputs(x, pos, wq_w, wq_b, wk_w, wk_b, wv_w, wv_b, wo_w, wo_b):
    x2 = np.asarray(x, np.float32).reshape(BT, D)
    xT = np.ascontiguousarray(x2.T).astype(BF16)

    pos1 = np.asarray(pos, np.float32).reshape(T)
    freq = (1.0 / 10000.0 ** (np.arange(0, DH, 2, np.float32) / DH)).astype(np.float32)
    ang = pos1[None, :] * freq[:, None]  # [64, T]
    cos1 = np.cos(ang).astype(BF16)
    sin1 = np.sin(ang).astype(BF16)
    # duplicate across both dh halves -> [128, T]; sin carries the
    # rotate-half sign: rows 0:64 = -sin, rows 64:128 = +sin
    cosT = np.concatenate([cos1, cos1], axis=0)
    sinT = np.concatenate([-sin1, sin1], axis=0)

    scale = np.float32(1.0 / np.sqrt(DH))

    def wslice(w, bvec, c, s=None, swap_row=False):
        w = np.asarray(w, np.float32)
        bvec = np.asarray(bvec, np.float32)
        ws = w[c * DOUT : (c + 1) * DOUT]  # [256, D]
        bs = bvec[c * DOUT : (c + 1) * DOUT]
        if s is not None:
            ws = ws * s
            bs = bs * s
        rows = D + 2 if swap_row else D + 1
        out = np.empty((rows, DOUT), BF16)
        out[:D] = ws.T.astype(BF16)
        out[D] = bs.astype(BF16)
        if swap_row:
            # per-head swapped dh halves of the bias
            bsw = bs.reshape(HL, 2, DH // 2)[:, ::-1, :].reshape(DOUT)
            out[D + 1] = bsw.astype(BF16)
        return out

    in_maps = []
    for c in range(N_CORES):
        woTc = (
            np.asarray(wo_w, np.float32)[:, c * DOUT : (c + 1) * DOUT]
            .T.astype(BF16)
            .copy()
        )
        in_maps.append(
            {
                "xT": xT,
                "wqT": wslice(wq_w, wq_b, c, scale, swap_row=True),
                "wkT": wslice(wk_w, wk_b, c, swap_row=True),
                "wvT": wslice(wv_w, wv_b, c, swap_row=True),
                "woT": woTc,
                "cosT": cosT,
                "sinT": sinT,
            }
        )
    return in_maps


def _run(in_maps, trace=False):
    _install_compile_patch()
    from concourse.bass_utils import run_bass_kernel_spmd

    nc = _get_nc()
    return run_bass_kernel_spmd(
        nc, in_maps, core_ids=list(range(N_CORES)), trace=trace
    )


def kernel(**inputs):
    inputs = {k: np.asarray(v) for k, v in inputs.items()}
    in_maps = _prep_inputs(**inputs)
    r = _run(in_maps, trace=False)
    acc = np.zeros((BT, D), np.float32)
    for c in range(N_CORES):
        acc += r.results[c]["out"].astype(np.float32)
    acc += np.asarray(inputs["wo_b"], np.float32)
    return acc.reshape(B, T, D)
